# revision 26
# baseline (speedup 1.0000x reference)
"""Causal single-head attention (B=4, T=4096, C=1024, D=64) on 8 NeuronCores.

Sharding: core c = (batch b = c % 4, half h = c // 4).
Each core handles ALL queries of its batch, but only its half of the key
blocks (256-token key blocks with block index ≡ h mod 2).  Pure SPMD; cores
differ only in input data.  Each core emits unnormalized partial results
U^T = [V|1]^T @ exp(S^T) per query supertile; the host combines the two
halves per batch: O = (U0 + U1)[:64] / (U0 + U1)[64].

Key trick: each 512-token chunk of x is column-permuted ON THE HOST so the
core's local key block is always the chunk's FIRST 256 columns (for h=1 the
two 256-blocks are swapped).  The program is h-independent; K/V projections
read the key region straight out of the xq tile (no separate xk input --
8MB instead of 12MB in), and the host un-permutes the h=1 output columns.

Performance structure:
  * All host tensors pre-laid-out partition-major/contiguous; one DMA per
    half-chunk.  Input split across both fast DGE queues (SP hi / ACT lo);
    outputs go via the GpSimd software queue.
  * Q projection uses duplicated weights [Wq|Wq] (M=128) -> qT2 holds Q^T
    in partitions 0-63 AND 64-127 (same cycles as M=64; matmul cost ~ N).
  * K/V projection fused as [Wv|Wk] (M=128): kvT partitions 0-63 = V^T
    (transpose-ready), 64-127 = K^T (odd-tile stationary, in place).
    K^T for even tiles is copied to kE partitions 0-63 by SBUF->SBUF DMA.
  * V' tiles by PE transpose of kvT[0:64] per 128-key tile into vP
    ([128, 65] per tile; col 64 = ones via one big memset).
  * S^T matmuls (contraction D=64) are row-tiled pairs: even key tile at
    tile_position (0,0), odd at (64,0) -> concurrent, ~2x S throughput.
    S/exp run one pair ahead of U so the strict-FIFO PE queue never
    stalls on the ACT exp.
  * exp on ACT covers both PSUM banks of a pair in one instruction.
"""
import sys
import numpy as np
import ml_dtypes

if "/opt/trn_rl_repo" not in sys.path:
    sys.path.insert(0, "/opt/trn_rl_repo")

import concourse.bacc as bacc
import concourse.mybir as mybir
from concourse import tile
from concourse import bass_utils

bf16 = mybir.dt.bfloat16
f32 = mybir.dt.float32
BF = ml_dtypes.bfloat16

B, T, C, D = 4, 4096, 1024, 64
NST = 8          # query supertiles per batch (512 queries each)
STQ = 512
TK = T // 2      # key tokens per core
NKT = TK // 128  # local 128-key tiles per core (16)
NC_ = C // 128   # 8 c-tiles

_CACHE = {}


def _build():
    nc = bacc.Bacc(None, target_bir_lowering=False, debug=False, num_devices=8)

    # host layout (st-major, fully contiguous per 512-query chunk, chunk
    # columns permuted so local keys are the first 256):
    #   xq[p, 4096*st + 512*c + t'] = x_perm^T[c*128 + p, 512*st + t']
    xq = nc.dram_tensor("xq", [128, NC_ * T], bf16, kind="ExternalInput")
    wq = nc.dram_tensor("wq", [128, NC_ * 128], bf16, kind="ExternalInput")
    wkv = nc.dram_tensor("wkv", [128, NC_ * 128], bf16, kind="ExternalInput")
    msk = nc.dram_tensor("msk", [128, 2 * STQ], bf16, kind="ExternalInput")
    idn = nc.dram_tensor("idn", [64, 64], bf16, kind="ExternalInput")
    out = nc.dram_tensor("out", [65, T], f32, kind="ExternalOutput")

    with tile.TileContext(nc) as tc:
        with tc.tile_pool(name="sb", bufs=1) as sb, \
             tc.tile_pool(name="pp", bufs=3) as pp, \
             tc.tile_pool(name="ps", bufs=2, space="PSUM") as ps:

            # ---- resident inputs ----
            xq_t = sb.tile([128, NC_ * T], bf16, tag="xq")
            wq_t = sb.tile([128, NC_ * 128], bf16, tag="wq")
            wkv_t = sb.tile([128, NC_ * 128], bf16, tag="wkv")
            msk_t = sb.tile([128, 2 * STQ], bf16, tag="msk")
            idn_t = sb.tile([64, 64], bf16, tag="idn")

            # split the weight halves across the two fast queues ahead of
            # chunk 0; masks/identity ride the (slow but idle) GpSimd queue
            nc.sync.dma_start(wq_t[:], wq[:])
            nc.scalar.dma_start(wkv_t[:], wkv[:])
            nc.gpsimd.dma_start(msk_t[:], msk[:])
            nc.gpsimd.dma_start(idn_t[:], idn[:])

            HQ = 4 * STQ   # half of a query chunk's columns
            # ACT carries the lo-halves of chunks 0-3 then is free to run the
            # exp stream; SP carries everything else (lo before hi for k>=4
            # so the chunk completes on the hi transfer)
            for st in range(4):
                x0 = 8 * STQ * st
                nc.sync.dma_start(xq_t[:, x0:x0 + HQ], xq[:, x0:x0 + HQ])
                nc.scalar.dma_start(xq_t[:, x0 + HQ:x0 + 2 * HQ],
                                    xq[:, x0 + HQ:x0 + 2 * HQ])
            for st in range(4, NST):
                x0 = 8 * STQ * st
                if st < 6:
                    nc.sync.dma_start(xq_t[:, x0 + HQ:x0 + 2 * HQ],
                                      xq[:, x0 + HQ:x0 + 2 * HQ])
                nc.sync.dma_start(xq_t[:, x0:x0 + HQ], xq[:, x0:x0 + HQ])

            # ---- persistent intermediates ----
            qT2 = sb.tile([128, T], bf16, tag="qT2")     # Q^T dup'd both halves
            kvT = sb.tile([128, TK], bf16, tag="kvT")    # p0:64 V^T, p64:128 K^T
            kE = sb.tile([128, TK], bf16, tag="kE")      # p0:64 = K^T (even tiles)
            vP = sb.tile([128, NKT * 65], bf16, tag="vP")  # V' tiles, col 64=ones

            nc.vector.memset(vP[:], 1.0)   # ones cols survive the transposes

            def s_pair(st, j0, qsl):
                """Row-tiled S matmul pair + exp (+ mask on the diagonal)."""
                n = 2 * (st + 1)
                s2 = ps.tile([128, 2 * STQ], f32, tag="s", name=f"s2_{st}_{j0}")
                p2 = pp.tile([128, 2 * STQ], bf16, tag="p", name=f"p2_{st}_{j0}")
                if st < 2:
                    # early supertiles: read K^T straight from kvT (both at
                    # row group (64,0), serial) -- removes the kE DMA from
                    # the kernel's critical start
                    nc.tensor.matmul(s2[:, 0:STQ],
                                     kvT[64:128, 128 * j0:128 * (j0 + 1)],
                                     qT2[64:128, qsl], start=True, stop=True)
                else:
                    nc.tensor.matmul(s2[:, 0:STQ],
                                     kE[0:64, 128 * j0:128 * (j0 + 1)],
                                     qT2[0:64, qsl], start=True, stop=True)
                nc.tensor.matmul(s2[:, STQ:2 * STQ],
                                 kvT[64:128, 128 * (j0 + 1):128 * (j0 + 2)],
                                 qT2[64:128, qsl], start=True, stop=True)
                nc.scalar.activation(p2[:], s2[:],
                                     mybir.ActivationFunctionType.Exp,
                                     scale=0.125)
                if j0 == n - 2:  # diagonal pair -> causal masks
                    nc.vector.tensor_mul(p2[:], p2[:], msk_t[:])
                return p2

            xq4 = xq_t[:].rearrange("p (s c n) -> p s c n", s=NST, c=NC_)

            def attention(st):
                qsl = slice(STQ * st, STQ * (st + 1))
                n = 2 * (st + 1)
                u = ps.tile([65, STQ], f32, tag="u", bufs=2, name=f"u{st}")
                p2_cur = s_pair(st, 0, qsl)
                for j0 in range(0, n, 2):
                    p2_nxt = s_pair(st, j0 + 2, qsl) if j0 + 2 < n else None
                    for dj in range(2):
                        j = j0 + dj
                        nc.tensor.matmul(u[:], vP[:, 65 * j:65 * (j + 1)],
                                         p2_cur[:, STQ * dj:STQ * (dj + 1)],
                                         start=(j == 0), stop=(j == n - 1))
                    p2_cur = p2_nxt
                u_sb = pp.tile([65, STQ], f32, tag="u_sb", bufs=8,
                               name=f"u_sb{st}")
                nc.vector.tensor_copy(u_sb[:], u[:])
                if st < 6:
                    nc.gpsimd.dma_start(out[:, qsl], u_sb[:])
                else:
                    nc.sync.dma_start(out[:, qsl], u_sb[:])

            for st in range(NST):
                # ---- Q projection (M=128: Wq|Wq) ----
                qsl = slice(STQ * st, STQ * (st + 1))
                accq = ps.tile([128, STQ], f32, tag="acc")
                for c in range(NC_):
                    xo = 8 * STQ * st + STQ * c
                    nc.tensor.matmul(accq[:], wq_t[:, 128 * c:128 * (c + 1)],
                                     xq_t[:, xo:xo + STQ],
                                     start=(c == 0), stop=(c == NC_ - 1))
                nc.vector.tensor_copy(qT2[:, qsl], accq[:])

                if st % 2 == 0:
                    continue
                g = st // 2
                k2sl = slice(512 * g, 512 * (g + 1))

                # ---- K/V projection (M=128: Wv|Wk) for chunk pair (2g, 2g+1)
                # keys = first 256 columns of each chunk -> N=512 via 3D AP.
                # Pair 0 is done per-chunk so attention(0) starts on chunk 0.
                if g < 2:
                    for half in range(2):
                        ch = 2 * g + half
                        acckv = ps.tile([128, STQ], f32, tag="acc")
                        for c in range(NC_):
                            nc.tensor.matmul(
                                acckv[:, 0:256],
                                wkv_t[:, 128 * c:128 * (c + 1)],
                                xq4[:, ch, c, 0:256],
                                start=(c == 0), stop=(c == NC_ - 1))
                        nc.vector.tensor_copy(
                            kvT[:, 256 * ch:256 * (ch + 1)],
                            acckv[:, 0:256])
                else:
                    acckv = ps.tile([128, STQ], f32, tag="acc")
                    for c in range(NC_):
                        nc.tensor.matmul(
                            acckv[:].rearrange("p (a n) -> p a n", a=2),
                            wkv_t[:, 128 * c:128 * (c + 1)],
                            xq4[:, 2 * g:2 * g + 2, c, 0:256],
                            start=(c == 0), stop=(c == NC_ - 1))
                    nc.vector.tensor_copy(kvT[:, k2sl], acckv[:])

                # K^T into partitions 0-63 for even row-tiles (fast ACT queue)
                nc.scalar.dma_start(kE[0:64, k2sl], kvT[64:128, k2sl])
                # V' tiles via PE transpose: [64,128] -> [128,64]
                for j in range(4 * g, 4 * g + 4):
                    tp = ps.tile([128, 64], bf16, tag="acc")
                    nc.tensor.transpose(tp[:], kvT[0:64, 128 * j:128 * (j + 1)],
                                        idn_t[:])
                    nc.vector.tensor_copy(vP[:, 65 * j:65 * j + 64], tp[:])

                attention(st - 1)
                attention(st)
                if st == 1:
                    # chunks 6/7 lo-halves on the now-idle ACT queue; emitted
                    # here so their dispatch doesn't delay the first exps
                    for lst in (6, 7):
                        x0 = 8 * STQ * lst
                        nc.scalar.dma_start(xq_t[:, x0 + HQ:x0 + 2 * HQ],
                                            xq[:, x0 + HQ:x0 + 2 * HQ])

    nc.compile()
    return nc


def _get_nc():
    if "nc" not in _CACHE:
        _CACHE["nc"] = _build()
    return _CACHE["nc"]


def kernel(x, Wq, Wk, Wv, _trace=False, _tmpdir=None):
    x = np.asarray(x)
    nc = _get_nc()

    xT = np.ascontiguousarray(x.transpose(0, 2, 1)).astype(BF)   # [B, C, T]
    wq_cat = np.concatenate([Wq, Wq], axis=1).astype(BF)         # [C, 128]
    wq_r = np.ascontiguousarray(
        wq_cat.reshape(NC_, 128, 128).transpose(1, 0, 2).reshape(128, NC_ * 128))
    wkv_cat = np.concatenate([Wv, Wk], axis=1).astype(BF)        # [C, 128]
    wkv_r = np.ascontiguousarray(
        wkv_cat.reshape(NC_, 128, 128).transpose(1, 0, 2).reshape(128, NC_ * 128))
    idn = np.eye(64, dtype=BF)

    # masks for the permuted chunk layout: query col i has in-chunk offset
    # qoff[i]; keys are the chunk's first 256 columns (local block).
    jj = np.arange(128)[:, None]
    masks = {}
    for h in range(2):
        if h == 0:
            qoff = np.arange(STQ)
        else:
            qoff = np.concatenate([np.arange(256), np.full(256, -1)])
            # permuted: cols 0-255 are the key block itself (offset >= key),
            # cols 256-511 are the EARLIER block -> never see these keys
        m0 = (jj <= qoff[None, :]).astype(BF)
        m1 = (jj + 128 <= qoff[None, :]).astype(BF)
        masks[h] = np.ascontiguousarray(np.concatenate([m0, m1], axis=1))

    in_maps = []
    for c in range(8):
        b, h = c % 4, c // 4
        xp = xT[b].reshape(C, NST, 2, 256)
        if h == 1:
            xp = xp[:, :, ::-1, :]                       # swap chunk halves
        xp = np.ascontiguousarray(xp).reshape(C, T)
        # st-major contiguous: xq_r[p, 4096*st + 512*c + t']
        xq_r = (xp.reshape(NC_, 128, NST, STQ)
                .transpose(1, 2, 0, 3).reshape(128, NC_ * T))
        in_maps.append({
            "xq": np.ascontiguousarray(xq_r),
            "wq": wq_r,
            "wkv": wkv_r,
            "msk": masks[h],
            "idn": idn,
        })

    res = bass_utils.run_bass_kernel_spmd(nc, in_maps, core_ids=list(range(8)),
                                          trace=_trace, tmpdir=_tmpdir)
    _CACHE["last_results"] = res

    O = np.empty((B, T, D), dtype=np.float32)
    for b in range(B):
        U0 = res.results[b]["out"]                       # h=0: natural order
        U1 = res.results[b + 4]["out"]                   # h=1: chunk-swapped
        U1 = np.ascontiguousarray(
            U1.reshape(65, NST, 2, 256)[:, :, ::-1, :]).reshape(65, T)
        U = U0 + U1
        O[b] = (U[:D] / U[D:D + 1]).T
    return O


# revision 27
# speedup vs baseline: 1.0212x; 1.0212x over previous
"""Causal single-head attention (B=4, T=4096, C=1024, D=64) on 8 NeuronCores.

Sharding: core c = (batch b = c % 4, half h = c // 4).
Each core handles ALL queries of its batch, but only its half of the key
blocks (256-token key blocks with block index ≡ h mod 2).  Pure SPMD; cores
differ only in input data.  Each core emits unnormalized partial results
U^T = [V|1]^T @ exp(S^T) per query supertile; the host combines the two
halves per batch: O = (U0 + U1)[:64] / (U0 + U1)[64].

Key trick: each 512-token chunk of x is column-permuted ON THE HOST so the
core's local key block is always the chunk's FIRST 256 columns (for h=1 the
two 256-blocks are swapped).  The program is h-independent; K/V projections
read the key region straight out of the xq tile (no separate xk input --
8MB instead of 12MB in), and the host un-permutes the h=1 output columns.

Performance structure:
  * All host tensors pre-laid-out partition-major/contiguous; one DMA per
    half-chunk.  Input split across both fast DGE queues (SP hi / ACT lo);
    outputs go via the GpSimd software queue.
  * Q projection uses duplicated weights [Wq|Wq] (M=128) -> qT2 holds Q^T
    in partitions 0-63 AND 64-127 (same cycles as M=64; matmul cost ~ N).
  * K/V projection fused as [Wv|Wk] (M=128): kvT partitions 0-63 = V^T
    (transpose-ready), 64-127 = K^T (odd-tile stationary, in place).
    K^T for even tiles is copied to kE partitions 0-63 by SBUF->SBUF DMA.
  * V' tiles by PE transpose of kvT[0:64] per 128-key tile into vP
    ([128, 65] per tile; col 64 = ones via one big memset).
  * S^T matmuls (contraction D=64) are row-tiled pairs: even key tile at
    tile_position (0,0), odd at (64,0) -> concurrent, ~2x S throughput.
    S/exp run one pair ahead of U so the strict-FIFO PE queue never
    stalls on the ACT exp.
  * exp on ACT covers both PSUM banks of a pair in one instruction.
"""
import sys
import numpy as np
import ml_dtypes

if "/opt/trn_rl_repo" not in sys.path:
    sys.path.insert(0, "/opt/trn_rl_repo")

import concourse.bacc as bacc
import concourse.mybir as mybir
from concourse import tile
from concourse import bass_utils

bf16 = mybir.dt.bfloat16
f32 = mybir.dt.float32
BF = ml_dtypes.bfloat16

B, T, C, D = 4, 4096, 1024, 64
NST = 8          # query supertiles per batch (512 queries each)
STQ = 512
TK = T // 2      # key tokens per core
NKT = TK // 128  # local 128-key tiles per core (16)
NC_ = C // 128   # 8 c-tiles

_CACHE = {}


def _build():
    nc = bacc.Bacc(None, target_bir_lowering=False, debug=False, num_devices=8)

    # host layout (st-major, fully contiguous per 512-query chunk, chunk
    # columns permuted so local keys are the first 256):
    #   xq[p, 4096*st + 512*c + t'] = x_perm^T[c*128 + p, 512*st + t']
    xq = nc.dram_tensor("xq", [128, NC_ * T], bf16, kind="ExternalInput")
    wq = nc.dram_tensor("wq", [128, NC_ * 128], bf16, kind="ExternalInput")
    wkv = nc.dram_tensor("wkv", [128, NC_ * 128], bf16, kind="ExternalInput")
    msk = nc.dram_tensor("msk", [128, 2 * STQ], bf16, kind="ExternalInput")
    idn = nc.dram_tensor("idn", [64, 64], bf16, kind="ExternalInput")
    out = nc.dram_tensor("out", [65, T], f32, kind="ExternalOutput")

    with tile.TileContext(nc) as tc:
        with tc.tile_pool(name="sb", bufs=1) as sb, \
             tc.tile_pool(name="pp", bufs=3) as pp, \
             tc.tile_pool(name="ps", bufs=2, space="PSUM") as ps:

            # ---- resident inputs ----
            xq_t = sb.tile([128, NC_ * T], bf16, tag="xq")
            wq_t = sb.tile([128, NC_ * 128], bf16, tag="wq")
            wkv_t = sb.tile([128, NC_ * 128], bf16, tag="wkv")
            msk_t = sb.tile([128, 2 * STQ], bf16, tag="msk")
            idn_t = sb.tile([64, 64], bf16, tag="idn")

            # split the weight halves across the two fast queues ahead of
            # chunk 0; masks/identity ride the (slow but idle) GpSimd queue
            nc.sync.dma_start(wq_t[:], wq[:])
            nc.scalar.dma_start(wkv_t[:], wkv[:])
            nc.gpsimd.dma_start(msk_t[:], msk[:])
            nc.gpsimd.dma_start(idn_t[:], idn[:])

            HQ = 4 * STQ   # half of a query chunk's columns
            # ACT carries the lo-halves of chunks 0-3 then is free to run the
            # exp stream; SP carries everything else (lo before hi for k>=4
            # so the chunk completes on the hi transfer)
            for st in range(4):
                x0 = 8 * STQ * st
                nc.sync.dma_start(xq_t[:, x0:x0 + HQ], xq[:, x0:x0 + HQ])
                nc.scalar.dma_start(xq_t[:, x0 + HQ:x0 + 2 * HQ],
                                    xq[:, x0 + HQ:x0 + 2 * HQ])
            for st in range(4, NST):
                x0 = 8 * STQ * st
                if st < 6:
                    nc.sync.dma_start(xq_t[:, x0 + HQ:x0 + 2 * HQ],
                                      xq[:, x0 + HQ:x0 + 2 * HQ])
                nc.sync.dma_start(xq_t[:, x0:x0 + HQ], xq[:, x0:x0 + HQ])

            # ---- persistent intermediates ----
            qT2 = sb.tile([128, T], bf16, tag="qT2")     # Q^T dup'd both halves
            kvT = sb.tile([128, TK], bf16, tag="kvT")    # p0:64 V^T, p64:128 K^T
            kE = sb.tile([128, TK], bf16, tag="kE")      # p0:64 = K^T (even tiles)
            vP = sb.tile([128, NKT * 65], bf16, tag="vP")  # V' tiles, col 64=ones

            nc.vector.memset(vP[:], 1.0)   # ones cols survive the transposes

            def s_pair(st, j0, qsl):
                """Row-tiled S matmul pair + exp (+ mask on the diagonal)."""
                n = 2 * (st + 1)
                s2 = ps.tile([128, 2 * STQ], f32, tag="s", name=f"s2_{st}_{j0}")
                p2 = pp.tile([128, 2 * STQ], bf16, tag="p", name=f"p2_{st}_{j0}")
                if st < 2:
                    # early supertiles: read K^T straight from kvT (both at
                    # row group (64,0), serial) -- removes the kE DMA from
                    # the kernel's critical start
                    nc.tensor.matmul(s2[:, 0:STQ],
                                     kvT[64:128, 128 * j0:128 * (j0 + 1)],
                                     qT2[64:128, qsl], start=True, stop=True)
                else:
                    nc.tensor.matmul(s2[:, 0:STQ],
                                     kE[0:64, 128 * j0:128 * (j0 + 1)],
                                     qT2[0:64, qsl], start=True, stop=True)
                nc.tensor.matmul(s2[:, STQ:2 * STQ],
                                 kvT[64:128, 128 * (j0 + 1):128 * (j0 + 2)],
                                 qT2[64:128, qsl], start=True, stop=True)
                nc.scalar.activation(p2[:], s2[:],
                                     mybir.ActivationFunctionType.Exp,
                                     scale=0.125)
                if j0 == n - 2:  # diagonal pair -> causal masks
                    nc.vector.tensor_mul(p2[:], p2[:], msk_t[:])
                return p2

            xq4 = xq_t[:].rearrange("p (s c n) -> p s c n", s=NST, c=NC_)

            def attention(st):
                qsl = slice(STQ * st, STQ * (st + 1))
                n = 2 * (st + 1)
                u = ps.tile([65, STQ], f32, tag="u", bufs=2, name=f"u{st}")
                p2_cur = s_pair(st, 0, qsl)
                for j0 in range(0, n, 2):
                    p2_nxt = s_pair(st, j0 + 2, qsl) if j0 + 2 < n else None
                    for dj in range(2):
                        j = j0 + dj
                        nc.tensor.matmul(u[:], vP[:, 65 * j:65 * (j + 1)],
                                         p2_cur[:, STQ * dj:STQ * (dj + 1)],
                                         start=(j == 0), stop=(j == n - 1))
                    p2_cur = p2_nxt
                u_sb = pp.tile([65, STQ], f32, tag="u_sb", bufs=8,
                               name=f"u_sb{st}")
                nc.vector.tensor_copy(u_sb[:], u[:])
                if st < 6:
                    nc.gpsimd.dma_start(out[:, qsl], u_sb[:])
                else:
                    nc.sync.dma_start(out[:, qsl], u_sb[:])

            for st in range(NST):
                # ---- Q projection (M=128: Wq|Wq) ----
                qsl = slice(STQ * st, STQ * (st + 1))
                accq = ps.tile([128, STQ], f32, tag="acc")
                for c in range(NC_):
                    xo = 8 * STQ * st + STQ * c
                    nc.tensor.matmul(accq[:], wq_t[:, 128 * c:128 * (c + 1)],
                                     xq_t[:, xo:xo + STQ],
                                     start=(c == 0), stop=(c == NC_ - 1))
                nc.vector.tensor_copy(qT2[:, qsl], accq[:])

                # ---- K/V projection (M=128: Wv|Wk), keys = first 256 of chunk
                ksl = slice(256 * st, 256 * (st + 1))
                acckv = ps.tile([128, STQ], f32, tag="acc")
                for c in range(NC_):
                    nc.tensor.matmul(acckv[:, 0:256],
                                     wkv_t[:, 128 * c:128 * (c + 1)],
                                     xq4[:, st, c, 0:256],
                                     start=(c == 0), stop=(c == NC_ - 1))
                nc.vector.tensor_copy(kvT[:, ksl], acckv[:, 0:256])

                # K^T into partitions 0-63 for even row-tiles (fast ACT queue;
                # emitted right after its KV copy so the ACT FIFO wait is short)
                nc.scalar.dma_start(kE[0:64, ksl], kvT[64:128, ksl])
                # V' tiles via PE transpose: [64,128] -> [128,64]
                for dj in range(2):
                    j = 2 * st + dj
                    tp = ps.tile([128, 64], bf16, tag="acc")
                    nc.tensor.transpose(tp[:], kvT[0:64, 128 * j:128 * (j + 1)],
                                        idn_t[:])
                    nc.vector.tensor_copy(vP[:, 65 * j:65 * j + 64], tp[:])

                attention(st)
                if st == 1:
                    # chunks 6/7 lo-halves on the now-idle ACT queue; emitted
                    # here so their dispatch doesn't delay the first exps
                    for lst in (6, 7):
                        x0 = 8 * STQ * lst
                        nc.scalar.dma_start(xq_t[:, x0 + HQ:x0 + 2 * HQ],
                                            xq[:, x0 + HQ:x0 + 2 * HQ])

    nc.compile()
    return nc


def _get_nc():
    if "nc" not in _CACHE:
        _CACHE["nc"] = _build()
    return _CACHE["nc"]


def kernel(x, Wq, Wk, Wv, _trace=False, _tmpdir=None):
    x = np.asarray(x)
    nc = _get_nc()

    xT = np.ascontiguousarray(x.transpose(0, 2, 1)).astype(BF)   # [B, C, T]
    wq_cat = np.concatenate([Wq, Wq], axis=1).astype(BF)         # [C, 128]
    wq_r = np.ascontiguousarray(
        wq_cat.reshape(NC_, 128, 128).transpose(1, 0, 2).reshape(128, NC_ * 128))
    wkv_cat = np.concatenate([Wv, Wk], axis=1).astype(BF)        # [C, 128]
    wkv_r = np.ascontiguousarray(
        wkv_cat.reshape(NC_, 128, 128).transpose(1, 0, 2).reshape(128, NC_ * 128))
    idn = np.eye(64, dtype=BF)

    # masks for the permuted chunk layout: query col i has in-chunk offset
    # qoff[i]; keys are the chunk's first 256 columns (local block).
    jj = np.arange(128)[:, None]
    masks = {}
    for h in range(2):
        if h == 0:
            qoff = np.arange(STQ)
        else:
            qoff = np.concatenate([np.arange(256), np.full(256, -1)])
            # permuted: cols 0-255 are the key block itself (offset >= key),
            # cols 256-511 are the EARLIER block -> never see these keys
        m0 = (jj <= qoff[None, :]).astype(BF)
        m1 = (jj + 128 <= qoff[None, :]).astype(BF)
        masks[h] = np.ascontiguousarray(np.concatenate([m0, m1], axis=1))

    in_maps = []
    for c in range(8):
        b, h = c % 4, c // 4
        xp = xT[b].reshape(C, NST, 2, 256)
        if h == 1:
            xp = xp[:, :, ::-1, :]                       # swap chunk halves
        xp = np.ascontiguousarray(xp).reshape(C, T)
        # st-major contiguous: xq_r[p, 4096*st + 512*c + t']
        xq_r = (xp.reshape(NC_, 128, NST, STQ)
                .transpose(1, 2, 0, 3).reshape(128, NC_ * T))
        in_maps.append({
            "xq": np.ascontiguousarray(xq_r),
            "wq": wq_r,
            "wkv": wkv_r,
            "msk": masks[h],
            "idn": idn,
        })

    res = bass_utils.run_bass_kernel_spmd(nc, in_maps, core_ids=list(range(8)),
                                          trace=_trace, tmpdir=_tmpdir)
    _CACHE["last_results"] = res

    O = np.empty((B, T, D), dtype=np.float32)
    for b in range(B):
        U0 = res.results[b]["out"]                       # h=0: natural order
        U1 = res.results[b + 4]["out"]                   # h=1: chunk-swapped
        U1 = np.ascontiguousarray(
            U1.reshape(65, NST, 2, 256)[:, :, ::-1, :]).reshape(65, T)
        U = U0 + U1
        O[b] = (U[:D] / U[D:D + 1]).T
    return O
